# revision 9
# baseline (speedup 1.0000x reference)
"""BCMP layer (GNN message passing) on 8 Trainium2 NeuronCores.

Math (see harness reference):
    out = (ahat(x@WX) + bhat(bcf@WZ) + ahat(bhat(bcf@Walpha))) / 3
By linearity of ahat:  out = ahat(G)/3 + bhat(bcf@WZ)/3  with
    G = x@WX + bhat(bcf@Walpha)
    ahat(G) = d*segsum_dest(d[col]*G[col]) + d^2*G ,  d = deg^-1/2

Two SPMD launches over 8 cores (destination nodes sharded, 12500/core):
  Launch 1: per-core slice of GS = d*G (bf16) and R = (d/3)*GS + Zprime/3
  (f32).  x arrives pre-scaled by d (host folds the row scale), weights in
  bf16.  The bc-table gather uses the dma_gather custom DMA.  Stores are
  batched per 7-window group.
  Launch 2: edge phase.  Destination slots are packed into 98 windows of
  128 (LPT on in-degree).  Messages GS[col] are fetched per (7-window
  group, source bank) with dma_gather; per-(window,bank) block counts are
  the max over cores (not a global max), which trims descriptor padding.
  One-hot segment-sum matmuls accumulate each window in PSUM;
  out = (d/3)*agg + R.

The Q7 descriptor-generation rate (~8ns/descriptor, one cpu pair) is the
hard floor for the gather stream; everything else is hidden under it.
"""

import math

import numpy as np
import ml_dtypes

import concourse.bacc as bacc
import concourse.mybir as mybir
from concourse.tile import TileContext
from concourse.bass_utils import run_bass_kernel_spmd

N = 100000
E = 1600000
M = 1000
D = 128
NCORES = 8
NC = N // NCORES            # 12500 nodes per core
P = 128
NW = NC // P + (1 if NC % P else 0)   # 98 windows per core
SLOTS = NW * P              # 12544 slots per core
MPAD = 1024                 # bc rows padded to 8 tiles
NB = 2                      # source banks (signed int16 reach: +-32768)
BANK_BASE = (32768, 98304)  # bank base rows; idx = row - base (int16)
BANK_SPLIT = 65536          # rows >= split go to bank 1
GRP = 7                     # windows per gather group
NGRP = NW // GRP            # 14 groups
INV3 = 1.0 / 3.0
C1 = 2.0 ** -0.5

F32 = mybir.dt.float32
BF16 = mybir.dt.bfloat16
I16 = mybir.dt.int16
I32 = mybir.dt.int32
AOP = mybir.AluOpType
ACT = mybir.ActivationFunctionType
BF16NP = ml_dtypes.bfloat16

CORE_IDS = list(range(NCORES))

LAST_RESULTS = []           # test harness hook

_kernel_cache = {}


def _wrap16(vals, n):
    """Pack flat idx list (len n) into dma_gather's [128, n//16] int16 layout:
    flat i -> [i % 16, i // 16], replicated across the 8 groups of 16
    partitions."""
    lay = np.zeros((16, n // 16), np.int16)
    lay[np.arange(n) % 16, np.arange(n) // 16] = vals
    return np.tile(lay, (8, 1))


def _pack_slots(vec, pad_value, ncols):
    """[values] -> [P, ncols] with flat index col*128+p."""
    tmp = np.full(ncols * P, pad_value, dtype=vec.dtype)
    tmp[: len(vec)] = vec
    return np.ascontiguousarray(tmp.reshape(ncols, P).T)


def _build_launch1():
    nc = bacc.Bacc()
    xT = nc.declare_dram_parameter("xdT", [P, SLOTS], BF16, isOutput=False)
    WXp = nc.declare_dram_parameter("WX", [P, D], BF16, isOutput=False)
    WAp = nc.declare_dram_parameter("WA", [P, D], BF16, isOutput=False)
    WZp = nc.declare_dram_parameter("WZ", [P, D], BF16, isOutput=False)
    bcfT = nc.declare_dram_parameter("bcfT", [P, MPAD], BF16, isOutput=False)
    NI1 = GRP * P
    aidx = nc.declare_dram_parameter("aidx16", [P, NGRP * (NI1 // 16)], I16,
                                     isOutput=False)
    degp = nc.declare_dram_parameter("deg", [P, NW], F32, isOutput=False)
    dcntp = nc.declare_dram_parameter("dcnt", [P, 8], F32, isOutput=False)
    emaskp = nc.declare_dram_parameter("emask", [P, 8], F32, isOutput=False)
    GS = nc.declare_dram_parameter("GS", [SLOTS, D], BF16, isOutput=True)
    Rout = nc.declare_dram_parameter("R", [SLOTS, D], F32, isOutput=True)
    T = nc.dram_tensor("T", [MPAD, 2 * D], BF16)

    with TileContext(nc) as tc:
        with (
            tc.tile_pool(name="const", bufs=1) as cpool,
            tc.tile_pool(name="zb", bufs=8) as zbpool,
            tc.tile_pool(name="gz", bufs=2) as gzpool,
            tc.tile_pool(name="work", bufs=3) as wpool,
            tc.tile_pool(name="grp", bufs=2) as gpool,
            tc.tile_pool(name="psum", bufs=2, space="PSUM") as ppool,
        ):
            wx = cpool.tile([P, D], BF16)
            nc.sync.dma_start(out=wx[:], in_=WXp[:])
            wa = cpool.tile([P, D], BF16)
            nc.sync.dma_start(out=wa[:], in_=WAp[:])
            wz = cpool.tile([P, D], BF16)
            nc.sync.dma_start(out=wz[:], in_=WZp[:])
            bcf = cpool.tile([P, MPAD], BF16)
            nc.sync.dma_start(out=bcf[:], in_=bcfT[:])
            asb = cpool.tile([P, NGRP * (NI1 // 16)], I16)
            nc.sync.dma_start(out=asb[:], in_=aidx[:])
            deg = cpool.tile([P, NW], F32)
            nc.sync.dma_start(out=deg[:], in_=degp[:])
            dcnt = cpool.tile([P, 8], F32)
            nc.sync.dma_start(out=dcnt[:], in_=dcntp[:])
            emask = cpool.tile([P, 8], F32)
            nc.sync.dma_start(out=emask[:], in_=emaskp[:])
            xsb = cpool.tile([P, SLOTS], BF16)
            nc.sync.dma_start(out=xsb[:], in_=xT[:])

            rec = cpool.tile([P, NW], F32)
            nc.vector.reciprocal(rec[:], deg[:])
            dsb = cpool.tile([P, NW], F32)
            nc.scalar.activation(dsb[:], rec[:], ACT.Sqrt)       # d
            dd3 = cpool.tile([P, NW], F32)
            nc.vector.tensor_scalar_mul(dd3[:], dsb[:], INV3)    # d/3
            rcc = cpool.tile([P, 8], F32)
            nc.vector.reciprocal(rcc[:], dcnt[:])
            dcol = cpool.tile([P, 8], F32)
            nc.scalar.activation(dcol[:], rcc[:], ACT.Sqrt)      # dcol
            dcol3 = cpool.tile([P, 8], F32)
            nc.scalar.activation(dcol3[:], rcc[:], ACT.Sqrt, scale=1.0 / 9.0)
            dm1 = cpool.tile([P, 8], F32)
            nc.vector.tensor_scalar_mul(dm1[:], emask[:], C1 - 1.0)
            em2 = cpool.tile([P, 8], F32)
            nc.vector.tensor_scalar_mul(em2[:], emask[:], C1)

            # broadcaster tables: T[:, :D] = dcol*(bcf@Walpha);
            #                     T[:, D:] = (dcol/3)*(bcf@WZ)
            tzb_list = []
            tzzb_list = []
            for jj in range(8):
                pz = ppool.tile([P, D], F32, space="PSUM", tag="pz")
                nc.tensor.matmul(
                    out=pz[:], lhsT=bcf[:, jj * P:(jj + 1) * P], rhs=wa[:],
                    start=True, stop=True,
                )
                tzb = zbpool.tile([P, D], BF16, tag="tzb")
                nc.vector.tensor_scalar(
                    out=tzb[:], in0=pz[:], scalar1=dcol[:, jj:jj + 1],
                    scalar2=None, op0=AOP.mult,
                )
                nc.sync.dma_start(out=T[jj * P:(jj + 1) * P, 0:D], in_=tzb[:])
                pz2 = ppool.tile([P, D], F32, space="PSUM", tag="pz2")
                nc.tensor.matmul(
                    out=pz2[:], lhsT=bcf[:, jj * P:(jj + 1) * P], rhs=wz[:],
                    start=True, stop=True,
                )
                tzzb = zbpool.tile([P, D], BF16, tag="tzzb")
                nc.vector.tensor_scalar(
                    out=tzzb[:], in0=pz2[:], scalar1=dcol3[:, jj:jj + 1],
                    scalar2=None, op0=AOP.mult,
                )
                nc.sync.dma_start(out=T[jj * P:(jj + 1) * P, D:2 * D],
                                  in_=tzzb[:])
                tzb_list.append(tzb)
                tzzb_list.append(tzzb)

            # T is read back by dma_gather below; order explicitly since Tile
            # does not track raw DRAM tensors.
            tc.strict_bb_all_engine_barrier()

            for g in range(NGRP):
                gz = gzpool.tile([P, GRP * 2 * D], BF16)
                nc.gpsimd.dma_gather(
                    out_ap=gz[:].rearrange("p (c r) -> p c r", c=GRP),
                    in_ap=T[:, :],
                    idxs_ap=asb[:, g * (NI1 // 16):(g + 1) * (NI1 // 16)],
                    num_idxs=NI1, num_idxs_reg=NI1, elem_size=2 * D,
                    single_packet=False,
                )
                gs_grp = gpool.tile([P, GRP * D], BF16, tag="gs")
                rt_grp = gpool.tile([P, GRP * D], F32, tag="rt")
                for k in range(GRP):
                    j = g * GRP + k
                    zba = gz[:, k * 2 * D: k * 2 * D + D]
                    zzba = gz[:, k * 2 * D + D: (k + 1) * 2 * D]

                    px = ppool.tile([P, D], F32, space="PSUM", tag="px")
                    nc.tensor.matmul(
                        out=px[:], lhsT=xsb[:, j * P:(j + 1) * P], rhs=wx[:],
                        start=True, stop=True,
                    )
                    zd = wpool.tile([P, D], F32, tag="zd")
                    nc.vector.tensor_scalar(
                        out=zd[:], in0=zba, scalar1=dsb[:, j:j + 1],
                        scalar2=None, op0=AOP.mult,
                    )
                    rin = zzba
                    if j < 8:
                        f1 = wpool.tile([P, D], F32, tag="f1")
                        nc.vector.tensor_scalar(
                            out=f1[:], in0=zd[:], scalar1=dm1[:, j:j + 1],
                            scalar2=None, op0=AOP.mult,
                        )
                        f2 = wpool.tile([P, D], F32, tag="f2")
                        nc.vector.tensor_scalar(
                            out=f2[:], in0=tzb_list[j][:],
                            scalar1=em2[:, j:j + 1],
                            scalar2=None, op0=AOP.mult,
                        )
                        f2d = wpool.tile([P, D], F32, tag="f2d")
                        nc.vector.tensor_scalar(
                            out=f2d[:], in0=f2[:], scalar1=dsb[:, j:j + 1],
                            scalar2=None, op0=AOP.mult,
                        )
                        nc.vector.tensor_add(out=zd[:], in0=zd[:], in1=f1[:])
                        nc.vector.tensor_add(out=zd[:], in0=zd[:], in1=f2d[:])
                        rf1 = wpool.tile([P, D], F32, tag="rf1")
                        nc.vector.tensor_scalar(
                            out=rf1[:], in0=zzba, scalar1=dm1[:, j:j + 1],
                            scalar2=None, op0=AOP.mult,
                        )
                        rf2 = wpool.tile([P, D], F32, tag="rf2")
                        nc.vector.tensor_scalar(
                            out=rf2[:], in0=tzzb_list[j][:],
                            scalar1=em2[:, j:j + 1],
                            scalar2=None, op0=AOP.mult,
                        )
                        rin_t = wpool.tile([P, D], F32, tag="rin")
                        nc.vector.tensor_add(out=rin_t[:], in0=zzba,
                                             in1=rf1[:])
                        nc.vector.tensor_add(out=rin_t[:], in0=rin_t[:],
                                             in1=rf2[:])
                        rin = rin_t[:]

                    gsl = gs_grp[:, k * D:(k + 1) * D]
                    nc.vector.tensor_add(out=gsl, in0=px[:], in1=zd[:])
                    r1 = wpool.tile([P, D], F32, tag="r1")
                    nc.scalar.activation(r1[:], gsl, ACT.Copy,
                                         scale=dd3[:, j:j + 1])
                    nc.vector.tensor_add(
                        out=rt_grp[:, k * D:(k + 1) * D], in0=r1[:], in1=rin)

                nc.sync.dma_start(
                    out=GS[g * GRP * P:(g + 1) * GRP * P, :].rearrange(
                        "(c p) f -> p c f", p=P),
                    in_=gs_grp[:].rearrange("p (c f) -> p c f", c=GRP),
                )
                nc.sync.dma_start(
                    out=Rout[g * GRP * P:(g + 1) * GRP * P, :].rearrange(
                        "(c p) f -> p c f", p=P),
                    in_=rt_grp[:].rearrange("p (c f) -> p c f", c=GRP),
                )

    nc.compile()
    return nc


CALLS = [(0, range(0, 4)), (0, range(4, 7)), (1, range(0, 7))]


def _build_launch2(Bmat):
    """Bmat: [NW][NB] python ints — blocks per (window, bank)."""
    ni_gc = []           # num idxs per (group, call)
    for g in range(NGRP):
        ni_gc.append([
            int(sum(Bmat[g * GRP + wl][q] for wl in wls)) * P
            for q, wls in CALLS
        ])
    ncol = int(sum(Bmat[w][q] for w in range(NW) for q in range(NB)))
    tot_i16 = sum(sum(ni) for ni in ni_gc) // 16

    nc = bacc.Bacc()
    GSp = nc.declare_dram_parameter("GS", [N, D], BF16, isOutput=False)
    idxp = nc.declare_dram_parameter("idx16", [P, tot_i16], I16,
                                     isOutput=False)
    dlcp = nc.declare_dram_parameter("dloc", [P, ncol], F32, isOutput=False)
    degwp = nc.declare_dram_parameter("degw", [P, NW], F32, isOutput=False)
    Rwp = nc.declare_dram_parameter("Rw", [SLOTS, D], F32, isOutput=False)
    iotap = nc.declare_dram_parameter("iota", [P, D], BF16, isOutput=False)
    OUT = nc.declare_dram_parameter("OUT", [SLOTS, D], F32, isOutput=True)

    with TileContext(nc) as tc:
        with (
            tc.tile_pool(name="const", bufs=1) as cpool,
            tc.tile_pool(name="msg", bufs=3) as msgp,
            tc.tile_pool(name="seg", bufs=4) as segp,
            tc.tile_pool(name="grp", bufs=2) as gpool,
            tc.tile_pool(name="fin", bufs=3) as finp,
            tc.tile_pool(name="psum", bufs=8, space="PSUM") as ppool,
        ):
            idx = cpool.tile([P, tot_i16], I16)
            nc.sync.dma_start(out=idx[:], in_=idxp[:])
            iota = cpool.tile([P, D], BF16)
            nc.sync.dma_start(out=iota[:], in_=iotap[:])
            dloc = cpool.tile([P, ncol], F32)
            nc.sync.dma_start(out=dloc[:], in_=dlcp[:])
            degw = cpool.tile([P, NW], F32)
            nc.sync.dma_start(out=degw[:], in_=degwp[:])
            rec = cpool.tile([P, NW], F32)
            nc.vector.reciprocal(rec[:], degw[:])
            dsc = cpool.tile([P, NW], F32)
            nc.scalar.activation(dsc[:], rec[:], ACT.Sqrt, scale=1.0 / 9.0)

            nimax = [max(ni_gc[g][c] for g in range(NGRP))
                     for c in range(len(CALLS))]
            ioff = 0          # idx16 column cursor
            col = 0           # dloc column cursor
            for g in range(NGRP):
                rw_grp = gpool.tile([P, GRP * D], F32, tag="rw")
                nc.sync.dma_start(
                    out=rw_grp[:].rearrange("p (c f) -> p c f", c=GRP),
                    in_=Rwp[g * GRP * P:(g + 1) * GRP * P, :].rearrange(
                        "(c p) f -> p c f", p=P),
                )
                msgs = []
                for ci, (q, wls) in enumerate(CALLS):
                    ni = ni_gc[g][ci]
                    m = msgp.tile([P, nimax[ci]], BF16, tag=f"m{ci}")
                    base = BANK_BASE[q]
                    nc.gpsimd.dma_gather(
                        out_ap=m[:, :ni].rearrange(
                            "p (c r) -> p c r", c=ni // P),
                        in_ap=GSp[base:base + 1696, :],
                        idxs_ap=idx[:, ioff:ioff + ni // 16],
                        num_idxs=ni, num_idxs_reg=ni, elem_size=D,
                        single_packet=False,
                    )
                    ioff += ni // 16
                    msgs.append(m)

                out_grp = gpool.tile([P, GRP * D], F32, tag="og")
                for wl in range(GRP):
                    w = g * GRP + wl
                    nblk = sum(Bmat[w][q] for q in range(NB))
                    ps = ppool.tile([P, D], F32, space="PSUM")
                    b = 0
                    for ci, (q, wls) in enumerate(CALLS):
                        if wl not in wls:
                            continue
                        off_w = sum(Bmat[g * GRP + wl2][q]
                                    for wl2 in wls if wl2 < wl)
                        for k in range(Bmat[w][q]):
                            sg = segp.tile([P, D], BF16)
                            nc.vector.tensor_scalar(
                                out=sg[:], in0=iota[:],
                                scalar1=dloc[:, col:col + 1],
                                scalar2=None, op0=AOP.is_equal,
                            )
                            chunk = off_w + k
                            nc.tensor.matmul(
                                out=ps[:], lhsT=sg[:],
                                rhs=msgs[ci][:, chunk * D:(chunk + 1) * D],
                                start=(b == 0), stop=(b == nblk - 1),
                            )
                            col += 1
                            b += 1
                    o1 = finp.tile([P, D], F32, tag="o1")
                    nc.scalar.activation(o1[:], ps[:], ACT.Copy,
                                         scale=dsc[:, w:w + 1])
                    nc.vector.tensor_add(
                        out=out_grp[:, wl * D:(wl + 1) * D],
                        in0=o1[:], in1=rw_grp[:, wl * D:(wl + 1) * D])
                nc.sync.dma_start(
                    out=OUT[g * GRP * P:(g + 1) * GRP * P, :].rearrange(
                        "(c p) f -> p c f", p=P),
                    in_=out_grp[:].rearrange("p (c f) -> p c f", c=GRP),
                )

    nc.compile()
    return nc


def _get_kernels(Bkey, Bmat):
    if "l1" not in _kernel_cache:
        _kernel_cache["l1"] = _build_launch1()
    if ("l2", Bkey) not in _kernel_cache:
        _kernel_cache[("l2", Bkey)] = _build_launch2(Bmat)
    return _kernel_cache["l1"], _kernel_cache[("l2", Bkey)]


def _prep_core(c, row_s, col_s, bounds, deg):
    """Host integer work: window packing + per-(window,bank) edge segments."""
    import heapq

    lo, hi = bounds[c * NC], bounds[(c + 1) * NC]
    edest = row_s[lo:hi] - c * NC          # local dest node of each edge
    ecol = col_s[lo:hi]                    # global source node
    ideg = (bounds[c * NC + 1:(c + 1) * NC + 1]
            - bounds[c * NC:(c + 1) * NC])  # local in-degree

    # greedy LPT assignment of nodes to windows (<=128 nodes per window)
    nodeorder = np.argsort(-ideg, kind="stable")
    heap = [(0, w) for w in range(NW)]
    heapq.heapify(heap)
    slots_used = np.zeros(NW, dtype=np.int64)
    wwin = np.empty(NC, dtype=np.int64)
    wslot = np.empty(NC, dtype=np.int64)
    for n in nodeorder:
        while True:
            load, w = heapq.heappop(heap)
            if slots_used[w] < P:
                break
        wwin[n] = w
        wslot[n] = slots_used[w]
        slots_used[w] += 1
        heapq.heappush(heap, (load + int(ideg[n]), w))

    w_e = wwin[edest]
    s_e = wslot[edest]
    q_e = (ecol >= BANK_SPLIT).astype(np.int64)
    rel_e = (ecol - np.array(BANK_BASE)[q_e]).astype(np.int16)
    key = w_e * NB + q_e
    eorder = np.argsort(key, kind="stable")
    counts = np.bincount(key, minlength=NW * NB).reshape(NW, NB)
    off = np.concatenate([[0], np.cumsum(counts.ravel())])

    perm = np.full(SLOTS, -1, dtype=np.int64)
    perm[wwin * P + wslot] = np.arange(c * NC, (c + 1) * NC)

    degw_flat = np.ones(SLOTS, dtype=np.float32)
    valid = perm >= 0
    degw_flat[valid] = deg[perm[valid]].astype(np.float32)
    degw = np.ascontiguousarray(degw_flat.reshape(NW, P).T)

    return {
        "rel_s": rel_e[eorder], "s_s": s_e[eorder],
        "counts": counts, "off": off,
        "perm": perm,
        "degw": degw,
    }


def kernel(x, edge_index, bc_feature, bc_assignment, WX, WZ, Walpha):
    x = np.asarray(x, dtype=np.float32)
    edge_index = np.asarray(edge_index)
    bc_feature = np.asarray(bc_feature, dtype=np.float32)
    bc_assignment = np.asarray(bc_assignment)
    WX = np.asarray(WX, dtype=np.float32)
    WZ = np.asarray(WZ, dtype=np.float32)
    Walpha = np.asarray(Walpha, dtype=np.float32)

    row = edge_index[0].astype(np.int64)
    col = edge_index[1].astype(np.int64)
    assign = bc_assignment.astype(np.int64)

    deg = np.bincount(col, minlength=N).astype(np.int64) + 1
    cnt = np.bincount(assign, minlength=M).astype(np.int64) + 1
    dfull = (1.0 / np.sqrt(deg.astype(np.float64))).astype(np.float32)

    order = np.argsort(row, kind="stable")
    row_s = row[order]
    col_s = col[order]
    bounds = np.searchsorted(row_s, np.arange(N + 1))

    cores = [_prep_core(c, row_s, col_s, bounds, deg) for c in range(NCORES)]

    # per-(window,bank) block counts: max over cores
    cmax = np.max(np.stack([ci["counts"] for ci in cores]), axis=0)
    Bmat_np = np.maximum((cmax + P - 1) // P, 1).astype(np.int64)
    Bmat = [[int(Bmat_np[w, q]) for q in range(NB)] for w in range(NW)]
    Bkey = Bmat_np.tobytes()

    nc1, nc2 = _get_kernels(Bkey, Bmat)

    # ---------------- launch 1 ----------------
    NI1 = GRP * P
    bcfT = np.zeros((P, MPAD), dtype=np.float32)
    bcfT[:, :M] = bc_feature.T
    bcfT = bcfT.astype(BF16NP)
    dcnt_sb = _pack_slots(cnt.astype(np.float32), np.float32(1.0), 8)
    in_maps1 = []
    for c in range(NCORES):
        xd = x[c * NC:(c + 1) * NC] * dfull[c * NC:(c + 1) * NC][:, None]
        xTc = np.zeros((P, SLOTS), dtype=np.float32)
        xTc[:, :NC] = xd.T
        a_pad = np.zeros(SLOTS, dtype=np.int16)
        a_pad[:NC] = assign[c * NC:(c + 1) * NC].astype(np.int16)
        aidx16 = np.concatenate(
            [_wrap16(a_pad[g * NI1:(g + 1) * NI1], NI1) for g in range(NGRP)],
            axis=1,
        )
        deg_sb = _pack_slots(deg[c * NC:(c + 1) * NC].astype(np.float32),
                             np.float32(1.0), NW)
        em = np.zeros(MPAD, dtype=np.float32)
        gids = c * NC + np.arange(MPAD)
        em[gids < M] = 1.0
        emask_sb = np.ascontiguousarray(em.reshape(8, P).T)
        in_maps1.append({
            "xdT": xTc.astype(BF16NP),
            "WX": WX.astype(BF16NP), "WA": Walpha.astype(BF16NP),
            "WZ": WZ.astype(BF16NP),
            "bcfT": bcfT,
            "aidx16": aidx16,
            "deg": deg_sb,
            "dcnt": dcnt_sb,
            "emask": emask_sb,
        })

    res1 = run_bass_kernel_spmd(nc1, in_maps1, core_ids=CORE_IDS)
    LAST_RESULTS.clear()
    LAST_RESULTS.append(res1)

    GS = np.concatenate(
        [np.asarray(res1.results[c]["GS"])[:NC] for c in range(NCORES)], axis=0
    )
    GS = np.ascontiguousarray(GS.astype(BF16NP))

    # ---------------- launch 2 ----------------
    iota = np.tile(np.arange(D, dtype=np.float32), (P, 1)).astype(BF16NP)
    ncol = int(Bmat_np.sum())
    in_maps2 = []
    for c in range(NCORES):
        ci = cores[c]
        rel_s, s_s, counts, off = (ci["rel_s"], ci["s_s"], ci["counts"],
                                   ci["off"])

        # per-(w,q) padded segments: idx (pads 0) + slots (pads -1)
        seg_rel = {}
        seg_slot = {}
        for w in range(NW):
            for q in range(NB):
                seg = int(counts[w, q])
                o = off[w * NB + q]
                nblk = Bmat[w][q]
                pr = np.zeros(nblk * P, dtype=np.int16)
                pr[:seg] = rel_s[o:o + seg]
                sl = np.full(nblk * P, -1.0, dtype=np.float32)
                sl[:seg] = s_s[o:o + seg].astype(np.float32)
                if seg == nblk * P and pr[-1] < 0:
                    # trailing negatives get stripped by the ucode; swap the
                    # last edge with a non-negative-index one in this segment
                    j = int(np.argmax(pr >= 0))
                    assert pr[j] >= 0
                    pr[-1], pr[j] = pr[j], pr[-1]
                    sl[-1], sl[j] = sl[j], sl[-1]
                seg_rel[(w, q)] = pr
                seg_slot[(w, q)] = sl

        dloc_cols = []
        idx_chunks = []
        for g in range(NGRP):
            for q, wls in CALLS:
                flat = np.concatenate(
                    [seg_rel[(g * GRP + wl, q)] for wl in wls])
                idx_chunks.append(_wrap16(flat, len(flat)))
            for wl in range(GRP):
                w = g * GRP + wl
                for q in range(NB):
                    nblk = Bmat[w][q]
                    dloc_cols.append(
                        seg_slot[(w, q)].reshape(nblk, P).T)
        dloc_k = np.ascontiguousarray(np.concatenate(dloc_cols, axis=1))
        assert dloc_k.shape[1] == ncol
        idx16_all = np.concatenate(idx_chunks, axis=1)

        R_c = np.asarray(res1.results[c]["R"])[:NC]
        Rw = np.zeros((SLOTS, D), dtype=np.float32)
        valid = ci["perm"] >= 0
        Rw[valid] = R_c[ci["perm"][valid] - c * NC]

        in_maps2.append({
            "GS": GS,
            "idx16": idx16_all,
            "dloc": dloc_k,
            "degw": ci["degw"],
            "Rw": Rw,
            "iota": iota,
        })

    res2 = run_bass_kernel_spmd(nc2, in_maps2, core_ids=CORE_IDS)
    LAST_RESULTS.append(res2)

    out = np.empty((N, D), dtype=np.float32)
    for c in range(NCORES):
        ci = cores[c]
        valid = ci["perm"] >= 0
        out[ci["perm"][valid]] = np.asarray(res2.results[c]["OUT"])[valid]
    return out


# revision 10
# speedup vs baseline: 1.0041x; 1.0041x over previous
"""BCMP layer (GNN message passing) on 8 Trainium2 NeuronCores.

Math (see harness reference):
    out = (ahat(x@WX) + bhat(bcf@WZ) + ahat(bhat(bcf@Walpha))) / 3
By linearity of ahat:  out = ahat(G)/3 + bhat(bcf@WZ)/3  with
    G = x@WX + bhat(bcf@Walpha)
    ahat(G) = d*segsum_dest(d[col]*G[col]) + d^2*G ,  d = deg^-1/2

Two SPMD launches over 8 cores (destination nodes sharded, 12500/core):
  Launch 1: per-core slice of GS = d*G (bf16) and R = (d/3)*GS + Zprime/3
  (f32).  x arrives pre-scaled by d (host folds the row scale), weights in
  bf16.  The bc-table gather uses the dma_gather custom DMA.  Stores are
  batched per 7-window group.
  Launch 2: edge phase.  Destination slots are packed into 98 windows of
  128 (LPT on in-degree).  Messages GS[col] are fetched per (7-window
  group, source bank) with dma_gather; per-(window,bank) block counts are
  the max over cores (not a global max), which trims descriptor padding.
  One-hot segment-sum matmuls accumulate each window in PSUM;
  out = (d/3)*agg + R.

The Q7 descriptor-generation rate (~8ns/descriptor, one cpu pair) is the
hard floor for the gather stream; everything else is hidden under it.
"""

import math

import numpy as np
import ml_dtypes

import concourse.bacc as bacc
import concourse.mybir as mybir
from concourse.tile import TileContext
from concourse.bass_utils import run_bass_kernel_spmd

N = 100000
E = 1600000
M = 1000
D = 128
NCORES = 8
NC = N // NCORES            # 12500 nodes per core
P = 128
NW = NC // P + (1 if NC % P else 0)   # 98 windows per core
SLOTS = NW * P              # 12544 slots per core
MPAD = 1024                 # bc rows padded to 8 tiles
NB = 2                      # source banks (signed int16 reach: +-32768)
BANK_BASE = (32768, 98304)  # bank base rows; idx = row - base (int16)
BANK_SPLIT = 65536          # rows >= split go to bank 1
GRP = 7                     # windows per gather group
NGRP = NW // GRP            # 14 groups
INV3 = 1.0 / 3.0
C1 = 2.0 ** -0.5

F32 = mybir.dt.float32
BF16 = mybir.dt.bfloat16
I16 = mybir.dt.int16
I32 = mybir.dt.int32
AOP = mybir.AluOpType
ACT = mybir.ActivationFunctionType
BF16NP = ml_dtypes.bfloat16

CORE_IDS = list(range(NCORES))

LAST_RESULTS = []           # test harness hook

_kernel_cache = {}


def _wrap16(vals, n):
    """Pack flat idx list (len n) into dma_gather's [128, n//16] int16 layout:
    flat i -> [i % 16, i // 16], replicated across the 8 groups of 16
    partitions."""
    lay = np.zeros((16, n // 16), np.int16)
    lay[np.arange(n) % 16, np.arange(n) // 16] = vals
    return np.tile(lay, (8, 1))


def _pack_slots(vec, pad_value, ncols):
    """[values] -> [P, ncols] with flat index col*128+p."""
    tmp = np.full(ncols * P, pad_value, dtype=vec.dtype)
    tmp[: len(vec)] = vec
    return np.ascontiguousarray(tmp.reshape(ncols, P).T)


def _build_launch1():
    nc = bacc.Bacc()
    xT = nc.declare_dram_parameter("xdT", [P, SLOTS], BF16, isOutput=False)
    WXp = nc.declare_dram_parameter("WX", [P, D], BF16, isOutput=False)
    WAp = nc.declare_dram_parameter("WA", [P, D], BF16, isOutput=False)
    WZp = nc.declare_dram_parameter("WZ", [P, D], BF16, isOutput=False)
    bcfT = nc.declare_dram_parameter("bcfT", [P, MPAD], BF16, isOutput=False)
    NI1 = GRP * P
    aidx = nc.declare_dram_parameter("aidx16", [P, NGRP * (NI1 // 16)], I16,
                                     isOutput=False)
    degp = nc.declare_dram_parameter("deg", [P, NW], F32, isOutput=False)
    dcntp = nc.declare_dram_parameter("dcnt", [P, 8], F32, isOutput=False)
    emaskp = nc.declare_dram_parameter("emask", [P, 8], F32, isOutput=False)
    GS = nc.declare_dram_parameter("GS", [SLOTS, D], BF16, isOutput=True)
    Rout = nc.declare_dram_parameter("R", [SLOTS, D], F32, isOutput=True)
    T = nc.dram_tensor("T", [MPAD, 2 * D], BF16)

    with TileContext(nc) as tc:
        with (
            tc.tile_pool(name="const", bufs=1) as cpool,
            tc.tile_pool(name="zb", bufs=8) as zbpool,
            tc.tile_pool(name="gz", bufs=2) as gzpool,
            tc.tile_pool(name="work", bufs=3) as wpool,
            tc.tile_pool(name="grp", bufs=2) as gpool,
            tc.tile_pool(name="psum", bufs=2, space="PSUM") as ppool,
        ):
            wx = cpool.tile([P, D], BF16)
            nc.sync.dma_start(out=wx[:], in_=WXp[:])
            wa = cpool.tile([P, D], BF16)
            nc.sync.dma_start(out=wa[:], in_=WAp[:])
            wz = cpool.tile([P, D], BF16)
            nc.sync.dma_start(out=wz[:], in_=WZp[:])
            bcf = cpool.tile([P, MPAD], BF16)
            nc.sync.dma_start(out=bcf[:], in_=bcfT[:])
            asb = cpool.tile([P, NGRP * (NI1 // 16)], I16)
            nc.sync.dma_start(out=asb[:], in_=aidx[:])
            deg = cpool.tile([P, NW], F32)
            nc.sync.dma_start(out=deg[:], in_=degp[:])
            dcnt = cpool.tile([P, 8], F32)
            nc.sync.dma_start(out=dcnt[:], in_=dcntp[:])
            emask = cpool.tile([P, 8], F32)
            nc.sync.dma_start(out=emask[:], in_=emaskp[:])
            xsb = cpool.tile([P, SLOTS], BF16)
            nc.sync.dma_start(out=xsb[:], in_=xT[:])

            rec = cpool.tile([P, NW], F32)
            nc.vector.reciprocal(rec[:], deg[:])
            dsb = cpool.tile([P, NW], F32)
            nc.scalar.activation(dsb[:], rec[:], ACT.Sqrt)       # d
            dd3 = cpool.tile([P, NW], F32)
            nc.vector.tensor_scalar_mul(dd3[:], dsb[:], INV3)    # d/3
            rcc = cpool.tile([P, 8], F32)
            nc.vector.reciprocal(rcc[:], dcnt[:])
            dcol = cpool.tile([P, 8], F32)
            nc.scalar.activation(dcol[:], rcc[:], ACT.Sqrt)      # dcol
            dcol3 = cpool.tile([P, 8], F32)
            nc.scalar.activation(dcol3[:], rcc[:], ACT.Sqrt, scale=1.0 / 9.0)
            dm1 = cpool.tile([P, 8], F32)
            nc.vector.tensor_scalar_mul(dm1[:], emask[:], C1 - 1.0)
            em2 = cpool.tile([P, 8], F32)
            nc.vector.tensor_scalar_mul(em2[:], emask[:], C1)

            # broadcaster tables: T[:, :D] = dcol*(bcf@Walpha);
            #                     T[:, D:] = (dcol/3)*(bcf@WZ)
            tzb_list = []
            tzzb_list = []
            for jj in range(8):
                pz = ppool.tile([P, D], F32, space="PSUM", tag="pz")
                nc.tensor.matmul(
                    out=pz[:], lhsT=bcf[:, jj * P:(jj + 1) * P], rhs=wa[:],
                    start=True, stop=True,
                )
                tzb = zbpool.tile([P, D], BF16, tag="tzb")
                nc.vector.tensor_scalar(
                    out=tzb[:], in0=pz[:], scalar1=dcol[:, jj:jj + 1],
                    scalar2=None, op0=AOP.mult,
                )
                nc.sync.dma_start(out=T[jj * P:(jj + 1) * P, 0:D], in_=tzb[:])
                pz2 = ppool.tile([P, D], F32, space="PSUM", tag="pz2")
                nc.tensor.matmul(
                    out=pz2[:], lhsT=bcf[:, jj * P:(jj + 1) * P], rhs=wz[:],
                    start=True, stop=True,
                )
                tzzb = zbpool.tile([P, D], BF16, tag="tzzb")
                nc.vector.tensor_scalar(
                    out=tzzb[:], in0=pz2[:], scalar1=dcol3[:, jj:jj + 1],
                    scalar2=None, op0=AOP.mult,
                )
                nc.sync.dma_start(out=T[jj * P:(jj + 1) * P, D:2 * D],
                                  in_=tzzb[:])
                tzb_list.append(tzb)
                tzzb_list.append(tzzb)

            # T is read back by dma_gather below; order explicitly since Tile
            # does not track raw DRAM tensors.
            tc.strict_bb_all_engine_barrier()

            for g in range(NGRP):
                gz = gzpool.tile([P, GRP * 2 * D], BF16)
                nc.gpsimd.dma_gather(
                    out_ap=gz[:].rearrange("p (c r) -> p c r", c=GRP),
                    in_ap=T[:, :],
                    idxs_ap=asb[:, g * (NI1 // 16):(g + 1) * (NI1 // 16)],
                    num_idxs=NI1, num_idxs_reg=NI1, elem_size=2 * D,
                    single_packet=False,
                )
                gs_grp = gpool.tile([P, GRP * D], BF16, tag="gs")
                rt_grp = gpool.tile([P, GRP * D], F32, tag="rt")
                for k in range(GRP):
                    j = g * GRP + k
                    zba = gz[:, k * 2 * D: k * 2 * D + D]
                    zzba = gz[:, k * 2 * D + D: (k + 1) * 2 * D]

                    px = ppool.tile([P, D], F32, space="PSUM", tag="px")
                    nc.tensor.matmul(
                        out=px[:], lhsT=xsb[:, j * P:(j + 1) * P], rhs=wx[:],
                        start=True, stop=True,
                    )
                    zd = wpool.tile([P, D], F32, tag="zd")
                    nc.vector.tensor_scalar(
                        out=zd[:], in0=zba, scalar1=dsb[:, j:j + 1],
                        scalar2=None, op0=AOP.mult,
                    )
                    rin = zzba
                    if j < 8:
                        f1 = wpool.tile([P, D], F32, tag="f1")
                        nc.vector.tensor_scalar(
                            out=f1[:], in0=zd[:], scalar1=dm1[:, j:j + 1],
                            scalar2=None, op0=AOP.mult,
                        )
                        f2 = wpool.tile([P, D], F32, tag="f2")
                        nc.vector.tensor_scalar(
                            out=f2[:], in0=tzb_list[j][:],
                            scalar1=em2[:, j:j + 1],
                            scalar2=None, op0=AOP.mult,
                        )
                        f2d = wpool.tile([P, D], F32, tag="f2d")
                        nc.vector.tensor_scalar(
                            out=f2d[:], in0=f2[:], scalar1=dsb[:, j:j + 1],
                            scalar2=None, op0=AOP.mult,
                        )
                        nc.vector.tensor_add(out=zd[:], in0=zd[:], in1=f1[:])
                        nc.vector.tensor_add(out=zd[:], in0=zd[:], in1=f2d[:])
                        rf1 = wpool.tile([P, D], F32, tag="rf1")
                        nc.vector.tensor_scalar(
                            out=rf1[:], in0=zzba, scalar1=dm1[:, j:j + 1],
                            scalar2=None, op0=AOP.mult,
                        )
                        rf2 = wpool.tile([P, D], F32, tag="rf2")
                        nc.vector.tensor_scalar(
                            out=rf2[:], in0=tzzb_list[j][:],
                            scalar1=em2[:, j:j + 1],
                            scalar2=None, op0=AOP.mult,
                        )
                        rin_t = wpool.tile([P, D], F32, tag="rin")
                        nc.vector.tensor_add(out=rin_t[:], in0=zzba,
                                             in1=rf1[:])
                        nc.vector.tensor_add(out=rin_t[:], in0=rin_t[:],
                                             in1=rf2[:])
                        rin = rin_t[:]

                    gsl = gs_grp[:, k * D:(k + 1) * D]
                    nc.vector.tensor_add(out=gsl, in0=px[:], in1=zd[:])
                    r1 = wpool.tile([P, D], F32, tag="r1")
                    nc.scalar.activation(r1[:], gsl, ACT.Copy,
                                         scale=dd3[:, j:j + 1])
                    nc.vector.tensor_add(
                        out=rt_grp[:, k * D:(k + 1) * D], in0=r1[:], in1=rin)

                nc.sync.dma_start(
                    out=GS[g * GRP * P:(g + 1) * GRP * P, :].rearrange(
                        "(c p) f -> p c f", p=P),
                    in_=gs_grp[:].rearrange("p (c f) -> p c f", c=GRP),
                )
                nc.sync.dma_start(
                    out=Rout[g * GRP * P:(g + 1) * GRP * P, :].rearrange(
                        "(c p) f -> p c f", p=P),
                    in_=rt_grp[:].rearrange("p (c f) -> p c f", c=GRP),
                )

    nc.compile()
    return nc


CALLS = [(0, range(0, 4)), (0, range(4, 7)), (1, range(0, 7))]


def _build_launch2(Bmat):
    """Bmat: [NW][NB] python ints — blocks per (window, bank)."""
    ni_gc = []           # num idxs per (group, call)
    for g in range(NGRP):
        ni_gc.append([
            int(sum(Bmat[g * GRP + wl][q] for wl in wls)) * P
            for q, wls in CALLS
        ])
    ncol = int(sum(Bmat[w][q] for w in range(NW) for q in range(NB)))
    tot_i16 = sum(sum(ni) for ni in ni_gc) // 16

    nc = bacc.Bacc()
    GSp = nc.declare_dram_parameter("GS", [N, D], BF16, isOutput=False)
    idxp = nc.declare_dram_parameter("idx16", [P, tot_i16], I16,
                                     isOutput=False)
    dlcp = nc.declare_dram_parameter("dloc", [P, ncol], F32, isOutput=False)
    degwp = nc.declare_dram_parameter("degw", [P, NW], F32, isOutput=False)
    Rwp = nc.declare_dram_parameter("Rw", [SLOTS, D], F32, isOutput=False)
    iotap = nc.declare_dram_parameter("iota", [P, D], BF16, isOutput=False)
    OUT = nc.declare_dram_parameter("OUT", [SLOTS, D], F32, isOutput=True)

    with TileContext(nc) as tc:
        with (
            tc.tile_pool(name="const", bufs=1) as cpool,
            tc.tile_pool(name="msg", bufs=2) as msgp,
            tc.tile_pool(name="seg", bufs=4) as segp,
            tc.tile_pool(name="grp", bufs=2) as gpool,
            tc.tile_pool(name="fin", bufs=3) as finp,
            tc.tile_pool(name="psum", bufs=8, space="PSUM") as ppool,
        ):
            idx = cpool.tile([P, tot_i16], I16)
            nc.sync.dma_start(out=idx[:], in_=idxp[:])
            iota = cpool.tile([P, D], BF16)
            nc.sync.dma_start(out=iota[:], in_=iotap[:])
            dloc = cpool.tile([P, ncol], F32)
            nc.sync.dma_start(out=dloc[:], in_=dlcp[:])
            degw = cpool.tile([P, NW], F32)
            nc.sync.dma_start(out=degw[:], in_=degwp[:])
            rec = cpool.tile([P, NW], F32)
            nc.vector.reciprocal(rec[:], degw[:])
            dsc = cpool.tile([P, NW], F32)
            nc.scalar.activation(dsc[:], rec[:], ACT.Sqrt, scale=1.0 / 9.0)

            nimax = [max(ni_gc[g][c] for g in range(NGRP))
                     for c in range(len(CALLS))]
            ioff = 0          # idx16 column cursor
            col = 0           # dloc column cursor
            for g in range(NGRP):
                rw_grp = gpool.tile([P, GRP * D], F32, tag="rw")
                nc.sync.dma_start(
                    out=rw_grp[:].rearrange("p (c f) -> p c f", c=GRP),
                    in_=Rwp[g * GRP * P:(g + 1) * GRP * P, :].rearrange(
                        "(c p) f -> p c f", p=P),
                )
                msgs = []
                for ci, (q, wls) in enumerate(CALLS):
                    ni = ni_gc[g][ci]
                    m = msgp.tile([P, nimax[ci]], BF16, tag=f"m{ci}")
                    base = BANK_BASE[q]
                    nc.gpsimd.dma_gather(
                        out_ap=m[:, :ni].rearrange(
                            "p (c r) -> p c r", c=ni // P),
                        in_ap=GSp[base:base + 1696, :],
                        idxs_ap=idx[:, ioff:ioff + ni // 16],
                        num_idxs=ni, num_idxs_reg=ni, elem_size=D,
                        single_packet=False,
                    )
                    ioff += ni // 16
                    msgs.append(m)

                out_grp = gpool.tile([P, GRP * D], F32, tag="og")
                for wl in range(GRP):
                    w = g * GRP + wl
                    nblk = sum(Bmat[w][q] for q in range(NB))
                    ps = ppool.tile([P, D], F32, space="PSUM")
                    b = 0
                    for ci, (q, wls) in enumerate(CALLS):
                        if wl not in wls:
                            continue
                        off_w = sum(Bmat[g * GRP + wl2][q]
                                    for wl2 in wls if wl2 < wl)
                        for k in range(Bmat[w][q]):
                            sg = segp.tile([P, D], BF16)
                            nc.vector.tensor_scalar(
                                out=sg[:], in0=iota[:],
                                scalar1=dloc[:, col:col + 1],
                                scalar2=None, op0=AOP.is_equal,
                            )
                            chunk = off_w + k
                            nc.tensor.matmul(
                                out=ps[:], lhsT=sg[:],
                                rhs=msgs[ci][:, chunk * D:(chunk + 1) * D],
                                start=(b == 0), stop=(b == nblk - 1),
                            )
                            col += 1
                            b += 1
                    o1 = finp.tile([P, D], F32, tag="o1")
                    nc.scalar.activation(o1[:], ps[:], ACT.Copy,
                                         scale=dsc[:, w:w + 1])
                    nc.vector.tensor_add(
                        out=out_grp[:, wl * D:(wl + 1) * D],
                        in0=o1[:], in1=rw_grp[:, wl * D:(wl + 1) * D])
                nc.sync.dma_start(
                    out=OUT[g * GRP * P:(g + 1) * GRP * P, :].rearrange(
                        "(c p) f -> p c f", p=P),
                    in_=out_grp[:].rearrange("p (c f) -> p c f", c=GRP),
                )

    nc.compile()
    return nc


def _get_kernels(Bkey, Bmat):
    if "l1" not in _kernel_cache:
        _kernel_cache["l1"] = _build_launch1()
    if ("l2", Bkey) not in _kernel_cache:
        _kernel_cache[("l2", Bkey)] = _build_launch2(Bmat)
    return _kernel_cache["l1"], _kernel_cache[("l2", Bkey)]


def _prep_core(c, row_s, col_s, bounds, deg):
    """Host integer work: window packing + per-(window,bank) edge segments."""
    import heapq

    lo, hi = bounds[c * NC], bounds[(c + 1) * NC]
    edest = row_s[lo:hi] - c * NC          # local dest node of each edge
    ecol = col_s[lo:hi]                    # global source node
    ideg = (bounds[c * NC + 1:(c + 1) * NC + 1]
            - bounds[c * NC:(c + 1) * NC])  # local in-degree

    # greedy LPT assignment of nodes to windows (<=128 nodes per window)
    nodeorder = np.argsort(-ideg, kind="stable")
    heap = [(0, w) for w in range(NW)]
    heapq.heapify(heap)
    slots_used = np.zeros(NW, dtype=np.int64)
    wwin = np.empty(NC, dtype=np.int64)
    wslot = np.empty(NC, dtype=np.int64)
    for n in nodeorder:
        while True:
            load, w = heapq.heappop(heap)
            if slots_used[w] < P:
                break
        wwin[n] = w
        wslot[n] = slots_used[w]
        slots_used[w] += 1
        heapq.heappush(heap, (load + int(ideg[n]), w))

    w_e = wwin[edest]
    s_e = wslot[edest]
    q_e = (ecol >= BANK_SPLIT).astype(np.int64)
    rel_e = (ecol - np.array(BANK_BASE)[q_e]).astype(np.int16)
    key = w_e * NB + q_e
    eorder = np.argsort(key, kind="stable")
    counts = np.bincount(key, minlength=NW * NB).reshape(NW, NB)
    off = np.concatenate([[0], np.cumsum(counts.ravel())])

    perm = np.full(SLOTS, -1, dtype=np.int64)
    perm[wwin * P + wslot] = np.arange(c * NC, (c + 1) * NC)

    degw_flat = np.ones(SLOTS, dtype=np.float32)
    valid = perm >= 0
    degw_flat[valid] = deg[perm[valid]].astype(np.float32)
    degw = np.ascontiguousarray(degw_flat.reshape(NW, P).T)

    return {
        "rel_s": rel_e[eorder], "s_s": s_e[eorder],
        "counts": counts, "off": off,
        "perm": perm,
        "degw": degw,
    }


def kernel(x, edge_index, bc_feature, bc_assignment, WX, WZ, Walpha):
    x = np.asarray(x, dtype=np.float32)
    edge_index = np.asarray(edge_index)
    bc_feature = np.asarray(bc_feature, dtype=np.float32)
    bc_assignment = np.asarray(bc_assignment)
    WX = np.asarray(WX, dtype=np.float32)
    WZ = np.asarray(WZ, dtype=np.float32)
    Walpha = np.asarray(Walpha, dtype=np.float32)

    row = edge_index[0].astype(np.int64)
    col = edge_index[1].astype(np.int64)
    assign = bc_assignment.astype(np.int64)

    deg = np.bincount(col, minlength=N).astype(np.int64) + 1
    cnt = np.bincount(assign, minlength=M).astype(np.int64) + 1
    dfull = (1.0 / np.sqrt(deg.astype(np.float64))).astype(np.float32)

    order = np.argsort(row, kind="stable")
    row_s = row[order]
    col_s = col[order]
    bounds = np.searchsorted(row_s, np.arange(N + 1))

    cores = [_prep_core(c, row_s, col_s, bounds, deg) for c in range(NCORES)]

    # per-(window,bank) block counts: max over cores
    cmax = np.max(np.stack([ci["counts"] for ci in cores]), axis=0)
    Bmat_np = np.maximum((cmax + P - 1) // P, 1).astype(np.int64)
    Bmat = [[int(Bmat_np[w, q]) for q in range(NB)] for w in range(NW)]
    Bkey = Bmat_np.tobytes()

    nc1, nc2 = _get_kernels(Bkey, Bmat)

    # ---------------- launch 1 ----------------
    NI1 = GRP * P
    bcfT = np.zeros((P, MPAD), dtype=np.float32)
    bcfT[:, :M] = bc_feature.T
    bcfT = bcfT.astype(BF16NP)
    dcnt_sb = _pack_slots(cnt.astype(np.float32), np.float32(1.0), 8)
    in_maps1 = []
    for c in range(NCORES):
        xd = x[c * NC:(c + 1) * NC] * dfull[c * NC:(c + 1) * NC][:, None]
        xTc = np.zeros((P, SLOTS), dtype=np.float32)
        xTc[:, :NC] = xd.T
        a_pad = np.zeros(SLOTS, dtype=np.int16)
        a_pad[:NC] = assign[c * NC:(c + 1) * NC].astype(np.int16)
        aidx16 = np.concatenate(
            [_wrap16(a_pad[g * NI1:(g + 1) * NI1], NI1) for g in range(NGRP)],
            axis=1,
        )
        deg_sb = _pack_slots(deg[c * NC:(c + 1) * NC].astype(np.float32),
                             np.float32(1.0), NW)
        em = np.zeros(MPAD, dtype=np.float32)
        gids = c * NC + np.arange(MPAD)
        em[gids < M] = 1.0
        emask_sb = np.ascontiguousarray(em.reshape(8, P).T)
        in_maps1.append({
            "xdT": xTc.astype(BF16NP),
            "WX": WX.astype(BF16NP), "WA": Walpha.astype(BF16NP),
            "WZ": WZ.astype(BF16NP),
            "bcfT": bcfT,
            "aidx16": aidx16,
            "deg": deg_sb,
            "dcnt": dcnt_sb,
            "emask": emask_sb,
        })

    res1 = run_bass_kernel_spmd(nc1, in_maps1, core_ids=CORE_IDS)
    LAST_RESULTS.clear()
    LAST_RESULTS.append(res1)

    GS = np.concatenate(
        [np.asarray(res1.results[c]["GS"])[:NC] for c in range(NCORES)], axis=0
    )
    GS = np.ascontiguousarray(GS.astype(BF16NP))

    # ---------------- launch 2 ----------------
    iota = np.tile(np.arange(D, dtype=np.float32), (P, 1)).astype(BF16NP)
    ncol = int(Bmat_np.sum())
    in_maps2 = []
    for c in range(NCORES):
        ci = cores[c]
        rel_s, s_s, counts, off = (ci["rel_s"], ci["s_s"], ci["counts"],
                                   ci["off"])

        # per-(w,q) padded segments: idx (pads 0) + slots (pads -1)
        seg_rel = {}
        seg_slot = {}
        for w in range(NW):
            for q in range(NB):
                seg = int(counts[w, q])
                o = off[w * NB + q]
                nblk = Bmat[w][q]
                pr = np.zeros(nblk * P, dtype=np.int16)
                pr[:seg] = rel_s[o:o + seg]
                sl = np.full(nblk * P, -1.0, dtype=np.float32)
                sl[:seg] = s_s[o:o + seg].astype(np.float32)
                if seg == nblk * P and pr[-1] < 0:
                    # trailing negatives get stripped by the ucode; swap the
                    # last edge with a non-negative-index one in this segment
                    j = int(np.argmax(pr >= 0))
                    assert pr[j] >= 0
                    pr[-1], pr[j] = pr[j], pr[-1]
                    sl[-1], sl[j] = sl[j], sl[-1]
                seg_rel[(w, q)] = pr
                seg_slot[(w, q)] = sl

        dloc_cols = []
        idx_chunks = []
        for g in range(NGRP):
            for q, wls in CALLS:
                flat = np.concatenate(
                    [seg_rel[(g * GRP + wl, q)] for wl in wls])
                idx_chunks.append(_wrap16(flat, len(flat)))
            for wl in range(GRP):
                w = g * GRP + wl
                for q in range(NB):
                    nblk = Bmat[w][q]
                    dloc_cols.append(
                        seg_slot[(w, q)].reshape(nblk, P).T)
        dloc_k = np.ascontiguousarray(np.concatenate(dloc_cols, axis=1))
        assert dloc_k.shape[1] == ncol
        idx16_all = np.concatenate(idx_chunks, axis=1)

        R_c = np.asarray(res1.results[c]["R"])[:NC]
        Rw = np.zeros((SLOTS, D), dtype=np.float32)
        valid = ci["perm"] >= 0
        Rw[valid] = R_c[ci["perm"][valid] - c * NC]

        in_maps2.append({
            "GS": GS,
            "idx16": idx16_all,
            "dloc": dloc_k,
            "degw": ci["degw"],
            "Rw": Rw,
            "iota": iota,
        })

    res2 = run_bass_kernel_spmd(nc2, in_maps2, core_ids=CORE_IDS)
    LAST_RESULTS.append(res2)

    out = np.empty((N, D), dtype=np.float32)
    for c in range(NCORES):
        ci = cores[c]
        valid = ci["perm"] >= 0
        out[ci["perm"][valid]] = np.asarray(res2.results[c]["OUT"])[valid]
    return out


# revision 11
# speedup vs baseline: 1.0465x; 1.0422x over previous
"""BCMP layer (GNN message passing) on 8 Trainium2 NeuronCores.

Math (see harness reference):
    out = (ahat(x@WX) + bhat(bcf@WZ) + ahat(bhat(bcf@Walpha))) / 3
By linearity of ahat:  out = ahat(G)/3 + bhat(bcf@WZ)/3  with
    G = x@WX + bhat(bcf@Walpha)
    ahat(G) = d*segsum_dest(d[col]*G[col]) + d^2*G ,  d = deg^-1/2

Two SPMD launches over 8 cores (destination nodes sharded, 12500/core):
  Launch 1: per-core slice of GS = d*G (bf16) and R = (d/3)*GS + Zprime/3
  (f32).  x arrives pre-scaled by d (host folds the row scale), weights in
  bf16.  The bc-table gather uses the dma_gather custom DMA.  Stores are
  batched per 7-window group.
  Launch 2: edge phase.  Destination slots are packed into 98 windows of
  128 (LPT on in-degree).  Messages GS[col] are fetched per (7-window
  group, source bank) with dma_gather; per-(window,bank) block counts are
  the max over cores (not a global max), which trims descriptor padding.
  One-hot segment-sum matmuls accumulate each window in PSUM;
  out = (d/3)*agg + R.

The Q7 descriptor-generation rate (~8ns/descriptor, one cpu pair) is the
hard floor for the gather stream; everything else is hidden under it.
"""

import math

import numpy as np
import ml_dtypes

import concourse.bacc as bacc
import concourse.mybir as mybir
from concourse.tile import TileContext
from concourse.bass_utils import run_bass_kernel_spmd

N = 100000
E = 1600000
M = 1000
D = 128
NCORES = 8
NC = N // NCORES            # 12500 nodes per core
P = 128
NW = NC // P + (1 if NC % P else 0)   # 98 windows per core
SLOTS = NW * P              # 12544 slots per core
MPAD = 1024                 # bc rows padded to 8 tiles
NB = 4                      # source banks (int16 index reach)
BANK_BASE = (0, 25000, 50000, 75000)
BANK = 25000                # rows per bank
GRP = 7                     # windows per gather group
NGRP = NW // GRP            # 14 groups
INV3 = 1.0 / 3.0
C1 = 2.0 ** -0.5

F32 = mybir.dt.float32
BF16 = mybir.dt.bfloat16
I16 = mybir.dt.int16
I32 = mybir.dt.int32
AOP = mybir.AluOpType
ACT = mybir.ActivationFunctionType
BF16NP = ml_dtypes.bfloat16

CORE_IDS = list(range(NCORES))

LAST_RESULTS = []           # test harness hook

_kernel_cache = {}


def _wrap16(vals, n):
    """Pack flat idx list (len n) into dma_gather's [128, n//16] int16 layout:
    flat i -> [i % 16, i // 16], replicated across the 8 groups of 16
    partitions."""
    lay = np.zeros((16, n // 16), np.int16)
    lay[np.arange(n) % 16, np.arange(n) // 16] = vals
    return np.tile(lay, (8, 1))


def _pack_slots(vec, pad_value, ncols):
    """[values] -> [P, ncols] with flat index col*128+p."""
    tmp = np.full(ncols * P, pad_value, dtype=vec.dtype)
    tmp[: len(vec)] = vec
    return np.ascontiguousarray(tmp.reshape(ncols, P).T)


def _build_launch1():
    nc = bacc.Bacc()
    xT = nc.declare_dram_parameter("xdT", [P, SLOTS], BF16, isOutput=False)
    WXp = nc.declare_dram_parameter("WX", [P, D], BF16, isOutput=False)
    WAp = nc.declare_dram_parameter("WA", [P, D], BF16, isOutput=False)
    WZp = nc.declare_dram_parameter("WZ", [P, D], BF16, isOutput=False)
    bcfT = nc.declare_dram_parameter("bcfT", [P, MPAD], BF16, isOutput=False)
    NI1 = GRP * P
    aidx = nc.declare_dram_parameter("aidx16", [P, NGRP * (NI1 // 16)], I16,
                                     isOutput=False)
    degp = nc.declare_dram_parameter("deg", [P, NW], F32, isOutput=False)
    dcntp = nc.declare_dram_parameter("dcnt", [P, 8], F32, isOutput=False)
    emaskp = nc.declare_dram_parameter("emask", [P, 8], F32, isOutput=False)
    GS = nc.declare_dram_parameter("GS", [SLOTS, D], BF16, isOutput=True)
    Rout = nc.declare_dram_parameter("R", [SLOTS, D], F32, isOutput=True)
    T = nc.dram_tensor("T", [MPAD, 2 * D], BF16)

    with TileContext(nc) as tc:
        with (
            tc.tile_pool(name="const", bufs=1) as cpool,
            tc.tile_pool(name="zb", bufs=8) as zbpool,
            tc.tile_pool(name="gz", bufs=2) as gzpool,
            tc.tile_pool(name="work", bufs=3) as wpool,
            tc.tile_pool(name="grp", bufs=2) as gpool,
            tc.tile_pool(name="psum", bufs=2, space="PSUM") as ppool,
        ):
            wx = cpool.tile([P, D], BF16)
            nc.sync.dma_start(out=wx[:], in_=WXp[:])
            wa = cpool.tile([P, D], BF16)
            nc.sync.dma_start(out=wa[:], in_=WAp[:])
            wz = cpool.tile([P, D], BF16)
            nc.sync.dma_start(out=wz[:], in_=WZp[:])
            bcf = cpool.tile([P, MPAD], BF16)
            nc.sync.dma_start(out=bcf[:], in_=bcfT[:])
            asb = cpool.tile([P, NGRP * (NI1 // 16)], I16)
            nc.sync.dma_start(out=asb[:], in_=aidx[:])
            deg = cpool.tile([P, NW], F32)
            nc.sync.dma_start(out=deg[:], in_=degp[:])
            dcnt = cpool.tile([P, 8], F32)
            nc.sync.dma_start(out=dcnt[:], in_=dcntp[:])
            emask = cpool.tile([P, 8], F32)
            nc.sync.dma_start(out=emask[:], in_=emaskp[:])
            xsb = cpool.tile([P, SLOTS], BF16)
            nc.sync.dma_start(out=xsb[:], in_=xT[:])

            rec = cpool.tile([P, NW], F32)
            nc.vector.reciprocal(rec[:], deg[:])
            dsb = cpool.tile([P, NW], F32)
            nc.scalar.activation(dsb[:], rec[:], ACT.Sqrt)       # d
            dd3 = cpool.tile([P, NW], F32)
            nc.vector.tensor_scalar_mul(dd3[:], dsb[:], INV3)    # d/3
            rcc = cpool.tile([P, 8], F32)
            nc.vector.reciprocal(rcc[:], dcnt[:])
            dcol = cpool.tile([P, 8], F32)
            nc.scalar.activation(dcol[:], rcc[:], ACT.Sqrt)      # dcol
            dcol3 = cpool.tile([P, 8], F32)
            nc.scalar.activation(dcol3[:], rcc[:], ACT.Sqrt, scale=1.0 / 9.0)
            dm1 = cpool.tile([P, 8], F32)
            nc.vector.tensor_scalar_mul(dm1[:], emask[:], C1 - 1.0)
            em2 = cpool.tile([P, 8], F32)
            nc.vector.tensor_scalar_mul(em2[:], emask[:], C1)

            # broadcaster tables: T[:, :D] = dcol*(bcf@Walpha);
            #                     T[:, D:] = (dcol/3)*(bcf@WZ)
            tzb_list = []
            tzzb_list = []
            for jj in range(8):
                pz = ppool.tile([P, D], F32, space="PSUM", tag="pz")
                nc.tensor.matmul(
                    out=pz[:], lhsT=bcf[:, jj * P:(jj + 1) * P], rhs=wa[:],
                    start=True, stop=True,
                )
                tzb = zbpool.tile([P, D], BF16, tag="tzb")
                nc.vector.tensor_scalar(
                    out=tzb[:], in0=pz[:], scalar1=dcol[:, jj:jj + 1],
                    scalar2=None, op0=AOP.mult,
                )
                nc.sync.dma_start(out=T[jj * P:(jj + 1) * P, 0:D], in_=tzb[:])
                pz2 = ppool.tile([P, D], F32, space="PSUM", tag="pz2")
                nc.tensor.matmul(
                    out=pz2[:], lhsT=bcf[:, jj * P:(jj + 1) * P], rhs=wz[:],
                    start=True, stop=True,
                )
                tzzb = zbpool.tile([P, D], BF16, tag="tzzb")
                nc.vector.tensor_scalar(
                    out=tzzb[:], in0=pz2[:], scalar1=dcol3[:, jj:jj + 1],
                    scalar2=None, op0=AOP.mult,
                )
                nc.sync.dma_start(out=T[jj * P:(jj + 1) * P, D:2 * D],
                                  in_=tzzb[:])
                tzb_list.append(tzb)
                tzzb_list.append(tzzb)

            # T is read back by dma_gather below; order explicitly since Tile
            # does not track raw DRAM tensors.
            tc.strict_bb_all_engine_barrier()

            for g in range(NGRP):
                gz = gzpool.tile([P, GRP * 2 * D], BF16)
                nc.gpsimd.dma_gather(
                    out_ap=gz[:].rearrange("p (c r) -> p c r", c=GRP),
                    in_ap=T[:, :],
                    idxs_ap=asb[:, g * (NI1 // 16):(g + 1) * (NI1 // 16)],
                    num_idxs=NI1, num_idxs_reg=NI1, elem_size=2 * D,
                    single_packet=False,
                )
                gs_grp = gpool.tile([P, GRP * D], BF16, tag="gs")
                rt_grp = gpool.tile([P, GRP * D], F32, tag="rt")
                for k in range(GRP):
                    j = g * GRP + k
                    zba = gz[:, k * 2 * D: k * 2 * D + D]
                    zzba = gz[:, k * 2 * D + D: (k + 1) * 2 * D]

                    px = ppool.tile([P, D], F32, space="PSUM", tag="px")
                    nc.tensor.matmul(
                        out=px[:], lhsT=xsb[:, j * P:(j + 1) * P], rhs=wx[:],
                        start=True, stop=True,
                    )
                    zd = wpool.tile([P, D], F32, tag="zd")
                    nc.vector.tensor_scalar(
                        out=zd[:], in0=zba, scalar1=dsb[:, j:j + 1],
                        scalar2=None, op0=AOP.mult,
                    )
                    rin = zzba
                    if j < 8:
                        f1 = wpool.tile([P, D], F32, tag="f1")
                        nc.vector.tensor_scalar(
                            out=f1[:], in0=zd[:], scalar1=dm1[:, j:j + 1],
                            scalar2=None, op0=AOP.mult,
                        )
                        f2 = wpool.tile([P, D], F32, tag="f2")
                        nc.vector.tensor_scalar(
                            out=f2[:], in0=tzb_list[j][:],
                            scalar1=em2[:, j:j + 1],
                            scalar2=None, op0=AOP.mult,
                        )
                        f2d = wpool.tile([P, D], F32, tag="f2d")
                        nc.vector.tensor_scalar(
                            out=f2d[:], in0=f2[:], scalar1=dsb[:, j:j + 1],
                            scalar2=None, op0=AOP.mult,
                        )
                        nc.vector.tensor_add(out=zd[:], in0=zd[:], in1=f1[:])
                        nc.vector.tensor_add(out=zd[:], in0=zd[:], in1=f2d[:])
                        rf1 = wpool.tile([P, D], F32, tag="rf1")
                        nc.vector.tensor_scalar(
                            out=rf1[:], in0=zzba, scalar1=dm1[:, j:j + 1],
                            scalar2=None, op0=AOP.mult,
                        )
                        rf2 = wpool.tile([P, D], F32, tag="rf2")
                        nc.vector.tensor_scalar(
                            out=rf2[:], in0=tzzb_list[j][:],
                            scalar1=em2[:, j:j + 1],
                            scalar2=None, op0=AOP.mult,
                        )
                        rin_t = wpool.tile([P, D], F32, tag="rin")
                        nc.vector.tensor_add(out=rin_t[:], in0=zzba,
                                             in1=rf1[:])
                        nc.vector.tensor_add(out=rin_t[:], in0=rin_t[:],
                                             in1=rf2[:])
                        rin = rin_t[:]

                    gsl = gs_grp[:, k * D:(k + 1) * D]
                    nc.vector.tensor_add(out=gsl, in0=px[:], in1=zd[:])
                    r1 = wpool.tile([P, D], F32, tag="r1")
                    nc.scalar.activation(r1[:], gsl, ACT.Copy,
                                         scale=dd3[:, j:j + 1])
                    nc.vector.tensor_add(
                        out=rt_grp[:, k * D:(k + 1) * D], in0=r1[:], in1=rin)

                nc.sync.dma_start(
                    out=GS[g * GRP * P:(g + 1) * GRP * P, :].rearrange(
                        "(c p) f -> p c f", p=P),
                    in_=gs_grp[:].rearrange("p (c f) -> p c f", c=GRP),
                )
                nc.sync.dma_start(
                    out=Rout[g * GRP * P:(g + 1) * GRP * P, :].rearrange(
                        "(c p) f -> p c f", p=P),
                    in_=rt_grp[:].rearrange("p (c f) -> p c f", c=GRP),
                )

    nc.compile()
    return nc


CALLS = [(0, range(0, 7)), (1, range(0, 7)), (2, range(0, 7)),
         (3, range(0, 7))]


def _build_launch2(Bmat):
    """Bmat: [NW][NB] python ints — blocks per (window, bank)."""
    ni_gc = []           # num idxs per (group, call)
    for g in range(NGRP):
        ni_gc.append([
            int(sum(Bmat[g * GRP + wl][q] for wl in wls)) * P
            for q, wls in CALLS
        ])
    ncol = int(sum(Bmat[w][q] for w in range(NW) for q in range(NB)))
    tot_i16 = sum(sum(ni) for ni in ni_gc) // 16

    nc = bacc.Bacc()
    GSp = nc.declare_dram_parameter("GS", [N, D], BF16, isOutput=False)
    idxp = nc.declare_dram_parameter("idx16", [P, tot_i16], I16,
                                     isOutput=False)
    dlcp = nc.declare_dram_parameter("dloc", [P, ncol], F32, isOutput=False)
    degwp = nc.declare_dram_parameter("degw", [P, NW], F32, isOutput=False)
    Rwp = nc.declare_dram_parameter("Rw", [SLOTS, D], F32, isOutput=False)
    iotap = nc.declare_dram_parameter("iota", [P, D], BF16, isOutput=False)
    OUT = nc.declare_dram_parameter("OUT", [SLOTS, D], F32, isOutput=True)

    with TileContext(nc) as tc:
        with (
            tc.tile_pool(name="const", bufs=1) as cpool,
            tc.tile_pool(name="msg", bufs=2) as msgp,
            tc.tile_pool(name="seg", bufs=4) as segp,
            tc.tile_pool(name="grp", bufs=2) as gpool,
            tc.tile_pool(name="fin", bufs=3) as finp,
            tc.tile_pool(name="psum", bufs=8, space="PSUM") as ppool,
        ):
            idx = cpool.tile([P, tot_i16], I16)
            nc.sync.dma_start(out=idx[:], in_=idxp[:])
            iota = cpool.tile([P, D], BF16)
            nc.sync.dma_start(out=iota[:], in_=iotap[:])
            dloc = cpool.tile([P, ncol], F32)
            nc.sync.dma_start(out=dloc[:], in_=dlcp[:])
            degw = cpool.tile([P, NW], F32)
            nc.sync.dma_start(out=degw[:], in_=degwp[:])
            rec = cpool.tile([P, NW], F32)
            nc.vector.reciprocal(rec[:], degw[:])
            dsc = cpool.tile([P, NW], F32)
            nc.scalar.activation(dsc[:], rec[:], ACT.Sqrt, scale=1.0 / 9.0)

            nimax = [max(ni_gc[g][c] for g in range(NGRP))
                     for c in range(len(CALLS))]
            ioff = 0          # idx16 column cursor
            col = 0           # dloc column cursor
            for g in range(NGRP):
                rw_grp = gpool.tile([P, GRP * D], F32, tag="rw")
                nc.sync.dma_start(
                    out=rw_grp[:].rearrange("p (c f) -> p c f", c=GRP),
                    in_=Rwp[g * GRP * P:(g + 1) * GRP * P, :].rearrange(
                        "(c p) f -> p c f", p=P),
                )
                msgs = []
                for ci, (q, wls) in enumerate(CALLS):
                    ni = ni_gc[g][ci]
                    m = msgp.tile([P, nimax[ci]], BF16, tag=f"m{ci}")
                    base = BANK_BASE[q]
                    nc.gpsimd.dma_gather(
                        out_ap=m[:, :ni].rearrange(
                            "p (c r) -> p c r", c=ni // P),
                        in_ap=GSp[base:base + 1696, :],
                        idxs_ap=idx[:, ioff:ioff + ni // 16],
                        num_idxs=ni, num_idxs_reg=ni, elem_size=D,
                        single_packet=False,
                    )
                    ioff += ni // 16
                    msgs.append(m)

                out_grp = gpool.tile([P, GRP * D], F32, tag="og")
                for wl in range(GRP):
                    w = g * GRP + wl
                    nblk = sum(Bmat[w][q] for q in range(NB))
                    ps = ppool.tile([P, D], F32, space="PSUM")
                    b = 0
                    for ci, (q, wls) in enumerate(CALLS):
                        if wl not in wls:
                            continue
                        off_w = sum(Bmat[g * GRP + wl2][q]
                                    for wl2 in wls if wl2 < wl)
                        for k in range(Bmat[w][q]):
                            sg = segp.tile([P, D], BF16)
                            nc.vector.tensor_scalar(
                                out=sg[:], in0=iota[:],
                                scalar1=dloc[:, col:col + 1],
                                scalar2=None, op0=AOP.is_equal,
                            )
                            chunk = off_w + k
                            nc.tensor.matmul(
                                out=ps[:], lhsT=sg[:],
                                rhs=msgs[ci][:, chunk * D:(chunk + 1) * D],
                                start=(b == 0), stop=(b == nblk - 1),
                            )
                            col += 1
                            b += 1
                    o1 = finp.tile([P, D], F32, tag="o1")
                    nc.scalar.activation(o1[:], ps[:], ACT.Copy,
                                         scale=dsc[:, w:w + 1])
                    nc.vector.tensor_add(
                        out=out_grp[:, wl * D:(wl + 1) * D],
                        in0=o1[:], in1=rw_grp[:, wl * D:(wl + 1) * D])
                nc.sync.dma_start(
                    out=OUT[g * GRP * P:(g + 1) * GRP * P, :].rearrange(
                        "(c p) f -> p c f", p=P),
                    in_=out_grp[:].rearrange("p (c f) -> p c f", c=GRP),
                )

    nc.compile()
    return nc


def _get_kernels(Bkey, Bmat):
    if "l1" not in _kernel_cache:
        _kernel_cache["l1"] = _build_launch1()
    if ("l2", Bkey) not in _kernel_cache:
        _kernel_cache[("l2", Bkey)] = _build_launch2(Bmat)
    return _kernel_cache["l1"], _kernel_cache[("l2", Bkey)]


def _prep_core(c, row_s, col_s, bounds, deg):
    """Host integer work: window packing + per-(window,bank) edge segments."""
    import heapq

    lo, hi = bounds[c * NC], bounds[(c + 1) * NC]
    edest = row_s[lo:hi] - c * NC          # local dest node of each edge
    ecol = col_s[lo:hi]                    # global source node
    ideg = (bounds[c * NC + 1:(c + 1) * NC + 1]
            - bounds[c * NC:(c + 1) * NC])  # local in-degree

    # greedy LPT assignment of nodes to windows (<=128 nodes per window)
    nodeorder = np.argsort(-ideg, kind="stable")
    heap = [(0, w) for w in range(NW)]
    heapq.heapify(heap)
    slots_used = np.zeros(NW, dtype=np.int64)
    wwin = np.empty(NC, dtype=np.int64)
    wslot = np.empty(NC, dtype=np.int64)
    for n in nodeorder:
        while True:
            load, w = heapq.heappop(heap)
            if slots_used[w] < P:
                break
        wwin[n] = w
        wslot[n] = slots_used[w]
        slots_used[w] += 1
        heapq.heappush(heap, (load + int(ideg[n]), w))

    w_e = wwin[edest]
    s_e = wslot[edest]
    q_e = ecol // BANK
    rel_e = (ecol - q_e * BANK).astype(np.int16)
    key = w_e * NB + q_e
    eorder = np.argsort(key, kind="stable")
    counts = np.bincount(key, minlength=NW * NB).reshape(NW, NB)
    off = np.concatenate([[0], np.cumsum(counts.ravel())])

    perm = np.full(SLOTS, -1, dtype=np.int64)
    perm[wwin * P + wslot] = np.arange(c * NC, (c + 1) * NC)

    degw_flat = np.ones(SLOTS, dtype=np.float32)
    valid = perm >= 0
    degw_flat[valid] = deg[perm[valid]].astype(np.float32)
    degw = np.ascontiguousarray(degw_flat.reshape(NW, P).T)

    return {
        "rel_s": rel_e[eorder], "s_s": s_e[eorder],
        "counts": counts, "off": off,
        "perm": perm,
        "degw": degw,
    }


def kernel(x, edge_index, bc_feature, bc_assignment, WX, WZ, Walpha):
    x = np.asarray(x, dtype=np.float32)
    edge_index = np.asarray(edge_index)
    bc_feature = np.asarray(bc_feature, dtype=np.float32)
    bc_assignment = np.asarray(bc_assignment)
    WX = np.asarray(WX, dtype=np.float32)
    WZ = np.asarray(WZ, dtype=np.float32)
    Walpha = np.asarray(Walpha, dtype=np.float32)

    row = edge_index[0].astype(np.int64)
    col = edge_index[1].astype(np.int64)
    assign = bc_assignment.astype(np.int64)

    deg = np.bincount(col, minlength=N).astype(np.int64) + 1
    cnt = np.bincount(assign, minlength=M).astype(np.int64) + 1
    dfull = (1.0 / np.sqrt(deg.astype(np.float64))).astype(np.float32)

    order = np.argsort(row, kind="stable")
    row_s = row[order]
    col_s = col[order]
    bounds = np.searchsorted(row_s, np.arange(N + 1))

    cores = [_prep_core(c, row_s, col_s, bounds, deg) for c in range(NCORES)]

    # per-(window,bank) block counts: max over cores
    cmax = np.max(np.stack([ci["counts"] for ci in cores]), axis=0)
    Bmat_np = np.maximum((cmax + P - 1) // P, 1).astype(np.int64)
    Bmat = [[int(Bmat_np[w, q]) for q in range(NB)] for w in range(NW)]
    Bkey = Bmat_np.tobytes()

    nc1, nc2 = _get_kernels(Bkey, Bmat)

    # ---------------- launch 1 ----------------
    NI1 = GRP * P
    bcfT = np.zeros((P, MPAD), dtype=np.float32)
    bcfT[:, :M] = bc_feature.T
    bcfT = bcfT.astype(BF16NP)
    dcnt_sb = _pack_slots(cnt.astype(np.float32), np.float32(1.0), 8)
    in_maps1 = []
    for c in range(NCORES):
        xd = x[c * NC:(c + 1) * NC] * dfull[c * NC:(c + 1) * NC][:, None]
        xTc = np.zeros((P, SLOTS), dtype=np.float32)
        xTc[:, :NC] = xd.T
        a_pad = np.zeros(SLOTS, dtype=np.int16)
        a_pad[:NC] = assign[c * NC:(c + 1) * NC].astype(np.int16)
        aidx16 = np.concatenate(
            [_wrap16(a_pad[g * NI1:(g + 1) * NI1], NI1) for g in range(NGRP)],
            axis=1,
        )
        deg_sb = _pack_slots(deg[c * NC:(c + 1) * NC].astype(np.float32),
                             np.float32(1.0), NW)
        em = np.zeros(MPAD, dtype=np.float32)
        gids = c * NC + np.arange(MPAD)
        em[gids < M] = 1.0
        emask_sb = np.ascontiguousarray(em.reshape(8, P).T)
        in_maps1.append({
            "xdT": xTc.astype(BF16NP),
            "WX": WX.astype(BF16NP), "WA": Walpha.astype(BF16NP),
            "WZ": WZ.astype(BF16NP),
            "bcfT": bcfT,
            "aidx16": aidx16,
            "deg": deg_sb,
            "dcnt": dcnt_sb,
            "emask": emask_sb,
        })

    res1 = run_bass_kernel_spmd(nc1, in_maps1, core_ids=CORE_IDS)
    LAST_RESULTS.clear()
    LAST_RESULTS.append(res1)

    GS = np.concatenate(
        [np.asarray(res1.results[c]["GS"])[:NC] for c in range(NCORES)], axis=0
    )
    GS = np.ascontiguousarray(GS.astype(BF16NP))

    # ---------------- launch 2 ----------------
    iota = np.tile(np.arange(D, dtype=np.float32), (P, 1)).astype(BF16NP)
    ncol = int(Bmat_np.sum())
    in_maps2 = []
    for c in range(NCORES):
        ci = cores[c]
        rel_s, s_s, counts, off = (ci["rel_s"], ci["s_s"], ci["counts"],
                                   ci["off"])

        # per-(w,q) padded segments: idx (pads 0) + slots (pads -1)
        seg_rel = {}
        seg_slot = {}
        for w in range(NW):
            for q in range(NB):
                seg = int(counts[w, q])
                o = off[w * NB + q]
                nblk = Bmat[w][q]
                pr = np.zeros(nblk * P, dtype=np.int16)
                pr[:seg] = rel_s[o:o + seg]
                sl = np.full(nblk * P, -1.0, dtype=np.float32)
                sl[:seg] = s_s[o:o + seg].astype(np.float32)
                if seg == nblk * P and pr[-1] < 0:
                    # trailing negatives get stripped by the ucode; swap the
                    # last edge with a non-negative-index one in this segment
                    j = int(np.argmax(pr >= 0))
                    assert pr[j] >= 0
                    pr[-1], pr[j] = pr[j], pr[-1]
                    sl[-1], sl[j] = sl[j], sl[-1]
                seg_rel[(w, q)] = pr
                seg_slot[(w, q)] = sl

        dloc_cols = []
        idx_chunks = []
        for g in range(NGRP):
            for q, wls in CALLS:
                flat = np.concatenate(
                    [seg_rel[(g * GRP + wl, q)] for wl in wls])
                idx_chunks.append(_wrap16(flat, len(flat)))
            for wl in range(GRP):
                w = g * GRP + wl
                for q in range(NB):
                    nblk = Bmat[w][q]
                    dloc_cols.append(
                        seg_slot[(w, q)].reshape(nblk, P).T)
        dloc_k = np.ascontiguousarray(np.concatenate(dloc_cols, axis=1))
        assert dloc_k.shape[1] == ncol
        idx16_all = np.concatenate(idx_chunks, axis=1)

        R_c = np.asarray(res1.results[c]["R"])[:NC]
        Rw = np.zeros((SLOTS, D), dtype=np.float32)
        valid = ci["perm"] >= 0
        Rw[valid] = R_c[ci["perm"][valid] - c * NC]

        in_maps2.append({
            "GS": GS,
            "idx16": idx16_all,
            "dloc": dloc_k,
            "degw": ci["degw"],
            "Rw": Rw,
            "iota": iota,
        })

    res2 = run_bass_kernel_spmd(nc2, in_maps2, core_ids=CORE_IDS)
    LAST_RESULTS.append(res2)

    out = np.empty((N, D), dtype=np.float32)
    for c in range(NCORES):
        ci = cores[c]
        valid = ci["perm"] >= 0
        out[ci["perm"][valid]] = np.asarray(res2.results[c]["OUT"])[valid]
    return out
